# revision 20
# baseline (speedup 1.0000x reference)
"""BitLinear (ternary weight) inference kernel for Trainium2, 8-core SPMD.

Full-input contract: kernel(**inputs) takes the complete tensors and returns
the complete output. The batch dim (B=8) is sharded 1:1 onto the 8
NeuronCores; each core computes y[b] = x[b] @ (w_q * 2^s_exp)^T + bias as a
2048^3 bf16 matmul with fp32 accumulation.

Host prep (cheap, O(bytes)): fold the power-of-two per-channel scale into
the ternary weights (exact in bf16), pre-transpose both operands into the
PE's contraction-major layout, cast x to bf16, broadcast bias to [128, OUT].

Device schedule: the k-contraction is split into two passes (k-chunks 0..3
and 4..15). Pass A only needs 3 MiB of inputs, so the PE starts ~8us in;
its partial sums go to HBM via plain SWDGE stores, and pass B accumulates
on top with accum_op=add DMAs on the same SWDGE ring (FIFO-ordered, so the
read-modify-write is safe). Inputs stream on the Sync HWDGE ring meanwhile.
"""
import os

import ml_dtypes
import numpy as np

B, T, IN, OUT = 8, 2048, 2048, 2048
P = 128
NCORES = 8
NF = 512        # matmul free dim (one PSUM bank of fp32)
KA = 4          # k-chunks in pass A (first-pass dependency set = KA MiB won't gate PE)

last_exec_time_ns = None
_CACHE = {}


def _install_prof_shim():
    """Make antenv.axon_hooks importable so trace=True works under axon."""
    import sys
    import types

    if "antenv.axon_hooks" in sys.modules:
        return
    try:
        from trn_agent_boot.trn_boot import _ntff_profile_via_ctypes
    except ImportError:
        return
    hook = _ntff_profile_via_ctypes("/opt/axon/libaxon_pjrt.so")
    mod = types.ModuleType("antenv.axon_hooks")
    mod.get_axon_ntff_profile_hook = lambda: hook
    mod.set_axon_ntff_profile_hook = lambda h: None
    sys.modules["antenv.axon_hooks"] = mod


def _build():
    import concourse.bacc as bacc
    import concourse.mybir as mybir
    from concourse.tile import TileContext

    nc = bacc.Bacc()
    x = nc.dram_tensor("x", (IN, T), mybir.dt.bfloat16, kind="ExternalInput")
    w = nc.dram_tensor("w", (IN, OUT), mybir.dt.float8e4, kind="ExternalInput")
    bias = nc.dram_tensor("bias", (P, OUT), mybir.dt.float32, kind="ExternalInput")
    y = nc.dram_tensor("y", (T, OUT), mybir.dt.float32, kind="ExternalOutput")

    KT = IN // P    # contraction chunks
    TT = T // P     # output row tiles
    OC = OUT // NF  # psum banks per row tile

    HOUT = OUT // 2  # two psum tiles (2 banks each) per row tile

    with TileContext(nc) as tc:
        with tc.tile_pool(name="wp", bufs=1) as wp, \
             tc.tile_pool(name="xp", bufs=1) as xp, \
             tc.tile_pool(name="bp", bufs=1) as bp, \
             tc.tile_pool(name="op", bufs=4) as op_, \
             tc.tile_pool(name="ptp", bufs=1) as ptp, \
             tc.tile_pool(name="pp", bufs=4, space="PSUM") as pp:

            # Interleave w/x chunk loads k-wise so pass A's working set
            # (k < KA) lands first and the PE can start after ~2 MiB.
            # Later chunks load pairwise (>=1 MiB DMAs for efficiency).
            w_tiles = [None] * KT
            xT_tiles = [None] * KT
            bias_t = bp.tile([P, OUT], mybir.dt.float32, tag="bias")
            x3 = x.rearrange("(ko p) t -> p ko t", p=P)
            w3 = w.rearrange("(ko p) o -> p ko o", p=P)

            # HAM pre-warm: dummy matmuls on a never-written scratch tile
            # while the first loads are in flight, so the PE clock-gate is
            # already at 8/8 when the real matmuls start.
            warm_sb = bp.tile([P, NF], mybir.dt.bfloat16, tag="warm")
            nc.gpsimd.memset(warm_sb, 0.0)
            warm_ps = pp.tile([P, HOUT], mybir.dt.float32, tag="ps",
                              name="warmps")
            for i in range(10):
                nc.tensor.matmul(warm_ps[:, :NF], warm_sb[:, :P], warm_sb,
                                 start=(i == 0), stop=(i == 9))

            for k in range(KA):
                wt = wp.tile([P, OUT], mybir.dt.float8e4, tag=f"w{k}")
                xt = xp.tile([P, T], mybir.dt.bfloat16, tag=f"x{k}")
                nc.sync.dma_start(wt, w[k * P:(k + 1) * P, :])
                nc.sync.dma_start(xt, x[k * P:(k + 1) * P, :])
                w_tiles[k] = wt
                xT_tiles[k] = xt
            nc.sync.dma_start(bias_t, bias[:, :])
            for k in range(KA, KT, 2):
                wt2 = wp.tile([P, 2, OUT], mybir.dt.float8e4, tag=f"w{k}")
                nc.sync.dma_start(wt2, w3[:, k:k + 2, :])
                w_tiles[k] = wt2[:, 0]
                w_tiles[k + 1] = wt2[:, 1]
                xt2 = xp.tile([P, 2, T], mybir.dt.bfloat16, tag=f"x{k}")
                nc.sync.dma_start(xt2, x3[:, k:k + 2, :])
                xT_tiles[k] = xt2[:, 0]
                xT_tiles[k + 1] = xt2[:, 1]

            TSPLIT = 6       # row tiles 0..TSPLIT-1 two-pass (partials in SBUF)

            partial_tiles = [
                ptp.tile([P, OUT], mybir.dt.float32, tag=f"pt{j}", name=f"pt{j}")
                for j in range(TSPLIT)
            ]

            def do_tiles(tt_range, k_lo, k_hi, mode):
                # mode: "partial" = bias add into SBUF partial (no store),
                #       "accum" = add SBUF partial + store,
                #       "single" = bias add + store
                for tt in tt_range:
                    pss = [pp.tile([P, HOUT], mybir.dt.float32, tag="ps",
                                   name=f"ps{h}") for h in range(2)]
                    for k in range(k_lo, k_hi):
                        lhsT = xT_tiles[k][:, tt * P:(tt + 1) * P]
                        for oc in range(OC):
                            ps = pss[oc // 2]
                            lo = (oc % 2) * NF
                            nc.tensor.matmul(
                                ps[:, lo:lo + NF],
                                lhsT,
                                w_tiles[k][:, oc * NF:(oc + 1) * NF],
                                start=(k == k_lo),
                                stop=(k == k_hi - 1),
                            )
                    if mode == "partial":
                        ot = partial_tiles[tt]
                    else:
                        ot = op_.tile([P, OUT], mybir.dt.float32, tag="out")
                    if tt == TT - 1:
                        # last tile: chunk epilogue+store so the store of
                        # chunk q overlaps the add of chunk q+1 (short tail)
                        for q in range(OC):
                            sl = slice(q * NF, (q + 1) * NF)
                            psl = slice((q % 2) * NF, (q % 2) * NF + NF)
                            nc.vector.tensor_add(ot[:, sl], pss[q // 2][:, psl],
                                                 bias_t[:, sl])
                            nc.scalar.dma_start(y[tt * P:(tt + 1) * P, sl],
                                                ot[:, sl])
                        continue
                    for h in range(2):
                        sl = slice(h * HOUT, (h + 1) * HOUT)
                        if mode == "accum":
                            nc.vector.tensor_add(ot[:, sl], pss[h],
                                                 partial_tiles[tt][:, sl])
                        else:
                            nc.vector.tensor_add(ot[:, sl], pss[h], bias_t[:, sl])
                    if mode != "partial":
                        nc.scalar.dma_start(y[tt * P:(tt + 1) * P, :], ot)

            do_tiles(range(TSPLIT), 0, KA, "partial")
            # Interleave accum and single-pass tiles so the PE always has
            # runnable chunks while the tail of the input load streams in.
            for j in range(TT - TSPLIT):
                if j < TSPLIT:
                    do_tiles([j], KA, KT, "accum")
                do_tiles([TSPLIT + j], 0, KT, "single")

    nc.compile()
    return nc


def kernel(x, w_q, s_exp, bias):
    global last_exec_time_ns
    from concourse.bass_utils import run_bass_kernel_spmd

    x = np.asarray(x)
    w_q = np.asarray(w_q)
    s_exp = np.asarray(s_exp)
    bias = np.asarray(bias, dtype=np.float32)
    assert x.shape == (B, T, IN) and w_q.shape == (OUT, IN)

    # Fold the power-of-two per-output-channel scale into the ternary
    # weights: values are +-2^s or 0 with s in [-8, 0], exact in fp8e4m3
    # (2^-8 and 2^-9 are exact subnormals).
    scale = np.exp2(s_exp.astype(np.float32))
    w_scaled_t = (w_q.astype(np.float32) * scale[:, None]).T
    w_fp8 = np.ascontiguousarray(w_scaled_t).astype(ml_dtypes.float8_e4m3fn)
    assert np.array_equal(w_fp8.astype(np.float32), w_scaled_t), \
        "scaled ternary weights must be exact in fp8e4m3"
    bias_bcast = np.ascontiguousarray(
        np.broadcast_to(bias.astype(np.float32), (P, OUT)))
    # Contraction-major layout for the PE: x^T[b] = [IN, T], bf16.
    xT_bf16 = np.ascontiguousarray(
        x.astype(ml_dtypes.bfloat16).transpose(0, 2, 1))

    nc = _CACHE.get("nc")
    if nc is None:
        nc = _CACHE["nc"] = _build()

    in_maps = [
        {"x": xT_bf16[b], "w": w_fp8, "bias": bias_bcast} for b in range(B)
    ]

    trace = bool(int(os.environ.get("BITLIN_TRACE", "0")))
    if trace:
        _install_prof_shim()
    res = run_bass_kernel_spmd(nc, in_maps, list(range(NCORES)), trace=trace)
    last_exec_time_ns = res.exec_time_ns

    out = np.stack([res.results[b]["y"] for b in range(B)], axis=0)
    return out.astype(np.float32, copy=False)


# revision 21
# speedup vs baseline: 1.0138x; 1.0138x over previous
"""BitLinear (ternary weight) inference kernel for Trainium2, 8-core SPMD.

Full-input contract: kernel(**inputs) takes the complete tensors and returns
the complete output. The batch dim (B=8) is sharded 1:1 onto the 8
NeuronCores; each core computes y[b] = x[b] @ (w_q * 2^s_exp)^T + bias as a
2048^3 bf16 matmul with fp32 accumulation.

Host prep (cheap, O(bytes)): fold the power-of-two per-channel scale into
the ternary weights (exact in bf16), pre-transpose both operands into the
PE's contraction-major layout, cast x to bf16, broadcast bias to [128, OUT].

Device schedule: the k-contraction is split into two passes (k-chunks 0..3
and 4..15). Pass A only needs 3 MiB of inputs, so the PE starts ~8us in;
its partial sums go to HBM via plain SWDGE stores, and pass B accumulates
on top with accum_op=add DMAs on the same SWDGE ring (FIFO-ordered, so the
read-modify-write is safe). Inputs stream on the Sync HWDGE ring meanwhile.
"""
import os

import ml_dtypes
import numpy as np

B, T, IN, OUT = 8, 2048, 2048, 2048
P = 128
NCORES = 8
NF = 512        # matmul free dim (one PSUM bank of fp32)
KA = 4          # k-chunks in pass A (first-pass dependency set = KA MiB won't gate PE)

last_exec_time_ns = None
_CACHE = {}


def _install_prof_shim():
    """Make antenv.axon_hooks importable so trace=True works under axon."""
    import sys
    import types

    if "antenv.axon_hooks" in sys.modules:
        return
    try:
        from trn_agent_boot.trn_boot import _ntff_profile_via_ctypes
    except ImportError:
        return
    hook = _ntff_profile_via_ctypes("/opt/axon/libaxon_pjrt.so")
    mod = types.ModuleType("antenv.axon_hooks")
    mod.get_axon_ntff_profile_hook = lambda: hook
    mod.set_axon_ntff_profile_hook = lambda h: None
    sys.modules["antenv.axon_hooks"] = mod


def _build():
    import concourse.bacc as bacc
    import concourse.mybir as mybir
    from concourse.tile import TileContext

    nc = bacc.Bacc()
    x = nc.dram_tensor("x", (IN, T), mybir.dt.bfloat16, kind="ExternalInput")
    w = nc.dram_tensor("w", (IN, OUT), mybir.dt.float8e4, kind="ExternalInput")
    bias = nc.dram_tensor("bias", (P, OUT), mybir.dt.float32, kind="ExternalInput")
    y = nc.dram_tensor("y", (T, OUT), mybir.dt.float32, kind="ExternalOutput")

    KT = IN // P    # contraction chunks
    TT = T // P     # output row tiles
    OC = OUT // NF  # psum banks per row tile

    HOUT = OUT // 2  # two psum tiles (2 banks each) per row tile

    with TileContext(nc) as tc:
        with tc.tile_pool(name="wp", bufs=1) as wp, \
             tc.tile_pool(name="xp", bufs=1) as xp, \
             tc.tile_pool(name="bp", bufs=1) as bp, \
             tc.tile_pool(name="op", bufs=4) as op_, \
             tc.tile_pool(name="ptp", bufs=1) as ptp, \
             tc.tile_pool(name="pp", bufs=4, space="PSUM") as pp:

            # Interleave w/x chunk loads k-wise so pass A's working set
            # (k < KA) lands first and the PE can start after ~2 MiB.
            # Later chunks load pairwise (>=1 MiB DMAs for efficiency).
            w_tiles = [None] * KT
            xT_tiles = [None] * KT
            bias_t = bp.tile([P, OUT], mybir.dt.float32, tag="bias")
            x3 = x.rearrange("(ko p) t -> p ko t", p=P)
            w3 = w.rearrange("(ko p) o -> p ko o", p=P)

            for k in range(KA):
                wt = wp.tile([P, OUT], mybir.dt.float8e4, tag=f"w{k}")
                xt = xp.tile([P, T], mybir.dt.bfloat16, tag=f"x{k}")
                nc.sync.dma_start(wt, w[k * P:(k + 1) * P, :])
                nc.sync.dma_start(xt, x[k * P:(k + 1) * P, :])
                w_tiles[k] = wt
                xT_tiles[k] = xt
            nc.sync.dma_start(bias_t, bias[:, :])
            for k in range(KA, KT, 2):
                wt2 = wp.tile([P, 2, OUT], mybir.dt.float8e4, tag=f"w{k}")
                nc.sync.dma_start(wt2, w3[:, k:k + 2, :])
                w_tiles[k] = wt2[:, 0]
                w_tiles[k + 1] = wt2[:, 1]
                xt2 = xp.tile([P, 2, T], mybir.dt.bfloat16, tag=f"x{k}")
                nc.sync.dma_start(xt2, x3[:, k:k + 2, :])
                xT_tiles[k] = xt2[:, 0]
                xT_tiles[k + 1] = xt2[:, 1]

            TSPLIT = 6       # row tiles 0..TSPLIT-1 two-pass (partials in SBUF)

            partial_tiles = [
                ptp.tile([P, OUT], mybir.dt.float32, tag=f"pt{j}", name=f"pt{j}")
                for j in range(TSPLIT)
            ]

            def do_tiles(tt_range, k_lo, k_hi, mode):
                # mode: "partial" = bias add into SBUF partial (no store),
                #       "accum" = add SBUF partial + store,
                #       "single" = bias add + store
                for tt in tt_range:
                    pss = [pp.tile([P, HOUT], mybir.dt.float32, tag="ps",
                                   name=f"ps{h}") for h in range(2)]
                    for k in range(k_lo, k_hi):
                        lhsT = xT_tiles[k][:, tt * P:(tt + 1) * P]
                        for oc in range(OC):
                            ps = pss[oc // 2]
                            lo = (oc % 2) * NF
                            nc.tensor.matmul(
                                ps[:, lo:lo + NF],
                                lhsT,
                                w_tiles[k][:, oc * NF:(oc + 1) * NF],
                                start=(k == k_lo),
                                stop=(k == k_hi - 1),
                            )
                    if mode == "partial":
                        ot = partial_tiles[tt]
                    else:
                        ot = op_.tile([P, OUT], mybir.dt.float32, tag="out")
                    if tt == TT - 1:
                        # last tile: chunk epilogue+store so the store of
                        # chunk q overlaps the add of chunk q+1 (short tail)
                        for q in range(OC):
                            sl = slice(q * NF, (q + 1) * NF)
                            psl = slice((q % 2) * NF, (q % 2) * NF + NF)
                            nc.vector.tensor_add(ot[:, sl], pss[q // 2][:, psl],
                                                 bias_t[:, sl])
                            nc.scalar.dma_start(y[tt * P:(tt + 1) * P, sl],
                                                ot[:, sl])
                        continue
                    for h in range(2):
                        sl = slice(h * HOUT, (h + 1) * HOUT)
                        if mode == "accum":
                            nc.vector.tensor_add(ot[:, sl], pss[h],
                                                 partial_tiles[tt][:, sl])
                        else:
                            nc.vector.tensor_add(ot[:, sl], pss[h], bias_t[:, sl])
                    if mode != "partial":
                        nc.scalar.dma_start(y[tt * P:(tt + 1) * P, :], ot)

            do_tiles(range(TSPLIT), 0, KA, "partial")
            # Interleave accum and single-pass tiles so the PE always has
            # runnable chunks while the tail of the input load streams in.
            for j in range(TT - TSPLIT):
                if j < TSPLIT:
                    do_tiles([j], KA, KT, "accum")
                do_tiles([TSPLIT + j], 0, KT, "single")

    nc.compile()
    return nc


def kernel(x, w_q, s_exp, bias):
    global last_exec_time_ns
    from concourse.bass_utils import run_bass_kernel_spmd

    x = np.asarray(x)
    w_q = np.asarray(w_q)
    s_exp = np.asarray(s_exp)
    bias = np.asarray(bias, dtype=np.float32)
    assert x.shape == (B, T, IN) and w_q.shape == (OUT, IN)

    # Fold the power-of-two per-output-channel scale into the ternary
    # weights: values are +-2^s or 0 with s in [-8, 0], exact in fp8e4m3
    # (2^-8 and 2^-9 are exact subnormals).
    scale = np.exp2(s_exp.astype(np.float32))
    w_scaled_t = (w_q.astype(np.float32) * scale[:, None]).T
    w_fp8 = np.ascontiguousarray(w_scaled_t).astype(ml_dtypes.float8_e4m3fn)
    assert np.array_equal(w_fp8.astype(np.float32), w_scaled_t), \
        "scaled ternary weights must be exact in fp8e4m3"
    bias_bcast = np.ascontiguousarray(
        np.broadcast_to(bias.astype(np.float32), (P, OUT)))
    # Contraction-major layout for the PE: x^T[b] = [IN, T], bf16.
    xT_bf16 = np.ascontiguousarray(
        x.astype(ml_dtypes.bfloat16).transpose(0, 2, 1))

    nc = _CACHE.get("nc")
    if nc is None:
        nc = _CACHE["nc"] = _build()

    in_maps = [
        {"x": xT_bf16[b], "w": w_fp8, "bias": bias_bcast} for b in range(B)
    ]

    trace = bool(int(os.environ.get("BITLIN_TRACE", "0")))
    if trace:
        _install_prof_shim()
    res = run_bass_kernel_spmd(nc, in_maps, list(range(NCORES)), trace=trace)
    last_exec_time_ns = res.exec_time_ns

    out = np.stack([res.results[b]["y"] for b in range(B)], axis=0)
    return out.astype(np.float32, copy=False)


# revision 22
# speedup vs baseline: 1.0154x; 1.0016x over previous
"""BitLinear (ternary weight) inference kernel for Trainium2, 8-core SPMD.

Full-input contract: kernel(**inputs) takes the complete tensors and returns
the complete output. The batch dim (B=8) is sharded 1:1 onto the 8
NeuronCores; each core computes y[b] = x[b] @ (w_q * 2^s_exp)^T + bias as a
2048^3 bf16 matmul with fp32 accumulation.

Host prep (cheap, O(bytes)): fold the power-of-two per-channel scale into
the ternary weights (exact in bf16), pre-transpose both operands into the
PE's contraction-major layout, cast x to bf16, broadcast bias to [128, OUT].

Device schedule: the k-contraction is split into two passes (k-chunks 0..3
and 4..15). Pass A only needs 3 MiB of inputs, so the PE starts ~8us in;
its partial sums go to HBM via plain SWDGE stores, and pass B accumulates
on top with accum_op=add DMAs on the same SWDGE ring (FIFO-ordered, so the
read-modify-write is safe). Inputs stream on the Sync HWDGE ring meanwhile.
"""
import os

import ml_dtypes
import numpy as np

B, T, IN, OUT = 8, 2048, 2048, 2048
P = 128
NCORES = 8
NF = 512        # matmul free dim (one PSUM bank of fp32)
KA = 4          # k-chunks in pass A (first-pass dependency set = KA MiB won't gate PE)

last_exec_time_ns = None
_CACHE = {}


def _install_prof_shim():
    """Make antenv.axon_hooks importable so trace=True works under axon."""
    import sys
    import types

    if "antenv.axon_hooks" in sys.modules:
        return
    try:
        from trn_agent_boot.trn_boot import _ntff_profile_via_ctypes
    except ImportError:
        return
    hook = _ntff_profile_via_ctypes("/opt/axon/libaxon_pjrt.so")
    mod = types.ModuleType("antenv.axon_hooks")
    mod.get_axon_ntff_profile_hook = lambda: hook
    mod.set_axon_ntff_profile_hook = lambda h: None
    sys.modules["antenv.axon_hooks"] = mod


def _build():
    import concourse.bacc as bacc
    import concourse.mybir as mybir
    from concourse.tile import TileContext

    nc = bacc.Bacc()
    x = nc.dram_tensor("x", (IN, T), mybir.dt.bfloat16, kind="ExternalInput")
    w = nc.dram_tensor("w", (IN, OUT), mybir.dt.float8e4, kind="ExternalInput")
    bias = nc.dram_tensor("bias", (P, OUT), mybir.dt.float32, kind="ExternalInput")
    y = nc.dram_tensor("y", (T, OUT), mybir.dt.float32, kind="ExternalOutput")

    KT = IN // P    # contraction chunks
    TT = T // P     # output row tiles
    OC = OUT // NF  # psum banks per row tile

    HOUT = OUT // 2  # two psum tiles (2 banks each) per row tile

    with TileContext(nc) as tc:
        with tc.tile_pool(name="wp", bufs=1) as wp, \
             tc.tile_pool(name="xp", bufs=1) as xp, \
             tc.tile_pool(name="bp", bufs=1) as bp, \
             tc.tile_pool(name="op", bufs=4) as op_, \
             tc.tile_pool(name="ptp", bufs=1) as ptp, \
             tc.tile_pool(name="pp", bufs=4, space="PSUM") as pp:

            # Interleave w/x chunk loads k-wise so pass A's working set
            # (k < KA) lands first and the PE can start after ~2 MiB.
            # Later chunks load pairwise (>=1 MiB DMAs for efficiency).
            w_tiles = [None] * KT
            xT_tiles = [None] * KT
            bias_t = bp.tile([P, OUT], mybir.dt.float32, tag="bias")
            x3 = x.rearrange("(ko p) t -> p ko t", p=P)
            w3 = w.rearrange("(ko p) o -> p ko o", p=P)

            # HAM pre-warm: a short burst of dummy matmuls on a scratch tile
            # while the first loads are in flight, so the PE clock-gate is
            # near 8/8 when the real matmuls start. Uses one "ps" slot
            # briefly (released well before pass A needs its 4th buffer).
            warm_sb = bp.tile([P, NF], mybir.dt.bfloat16, tag="warm")
            nc.gpsimd.memset(warm_sb, 0.0)
            warm_ps = pp.tile([P, HOUT], mybir.dt.float32, tag="ps",
                              name="warmps")
            for i in range(6):
                nc.tensor.matmul(warm_ps[:, :NF], warm_sb[:, :P], warm_sb,
                                 start=(i == 0), stop=(i == 5))

            for k in range(KA):
                wt = wp.tile([P, OUT], mybir.dt.float8e4, tag=f"w{k}")
                xt = xp.tile([P, T], mybir.dt.bfloat16, tag=f"x{k}")
                nc.sync.dma_start(wt, w[k * P:(k + 1) * P, :])
                nc.sync.dma_start(xt, x[k * P:(k + 1) * P, :])
                w_tiles[k] = wt
                xT_tiles[k] = xt
            nc.sync.dma_start(bias_t, bias[:, :])
            for k in range(KA, KT, 2):
                wt2 = wp.tile([P, 2, OUT], mybir.dt.float8e4, tag=f"w{k}")
                nc.sync.dma_start(wt2, w3[:, k:k + 2, :])
                w_tiles[k] = wt2[:, 0]
                w_tiles[k + 1] = wt2[:, 1]
                xt2 = xp.tile([P, 2, T], mybir.dt.bfloat16, tag=f"x{k}")
                nc.sync.dma_start(xt2, x3[:, k:k + 2, :])
                xT_tiles[k] = xt2[:, 0]
                xT_tiles[k + 1] = xt2[:, 1]

            TSPLIT = 6       # row tiles 0..TSPLIT-1 two-pass (partials in SBUF)

            partial_tiles = [
                ptp.tile([P, OUT], mybir.dt.float32, tag=f"pt{j}", name=f"pt{j}")
                for j in range(TSPLIT)
            ]

            def do_tiles(tt_range, k_lo, k_hi, mode):
                # mode: "partial" = bias add into SBUF partial (no store),
                #       "accum" = add SBUF partial + store,
                #       "single" = bias add + store
                for tt in tt_range:
                    pss = [pp.tile([P, HOUT], mybir.dt.float32, tag="ps",
                                   name=f"ps{h}") for h in range(2)]
                    for k in range(k_lo, k_hi):
                        lhsT = xT_tiles[k][:, tt * P:(tt + 1) * P]
                        for oc in range(OC):
                            ps = pss[oc // 2]
                            lo = (oc % 2) * NF
                            nc.tensor.matmul(
                                ps[:, lo:lo + NF],
                                lhsT,
                                w_tiles[k][:, oc * NF:(oc + 1) * NF],
                                start=(k == k_lo),
                                stop=(k == k_hi - 1),
                            )
                    if mode == "partial":
                        ot = partial_tiles[tt]
                    else:
                        ot = op_.tile([P, OUT], mybir.dt.float32, tag="out")
                    if tt == TT - 1:
                        # last tile: chunk epilogue+store so the store of
                        # chunk q overlaps the add of chunk q+1 (short tail)
                        for q in range(OC):
                            sl = slice(q * NF, (q + 1) * NF)
                            psl = slice((q % 2) * NF, (q % 2) * NF + NF)
                            nc.vector.tensor_add(ot[:, sl], pss[q // 2][:, psl],
                                                 bias_t[:, sl])
                            nc.scalar.dma_start(y[tt * P:(tt + 1) * P, sl],
                                                ot[:, sl])
                        continue
                    for h in range(2):
                        sl = slice(h * HOUT, (h + 1) * HOUT)
                        if mode == "accum":
                            nc.vector.tensor_add(ot[:, sl], pss[h],
                                                 partial_tiles[tt][:, sl])
                        else:
                            nc.vector.tensor_add(ot[:, sl], pss[h], bias_t[:, sl])
                    if mode != "partial":
                        nc.scalar.dma_start(y[tt * P:(tt + 1) * P, :], ot)

            do_tiles(range(TSPLIT), 0, KA, "partial")
            # Interleave accum and single-pass tiles so the PE always has
            # runnable chunks while the tail of the input load streams in.
            for j in range(TT - TSPLIT):
                if j < TSPLIT:
                    do_tiles([j], KA, KT, "accum")
                do_tiles([TSPLIT + j], 0, KT, "single")

    nc.compile()
    return nc


def kernel(x, w_q, s_exp, bias):
    global last_exec_time_ns
    from concourse.bass_utils import run_bass_kernel_spmd

    x = np.asarray(x)
    w_q = np.asarray(w_q)
    s_exp = np.asarray(s_exp)
    bias = np.asarray(bias, dtype=np.float32)
    assert x.shape == (B, T, IN) and w_q.shape == (OUT, IN)

    # Fold the power-of-two per-output-channel scale into the ternary
    # weights: values are +-2^s or 0 with s in [-8, 0], exact in fp8e4m3
    # (2^-8 and 2^-9 are exact subnormals).
    scale = np.exp2(s_exp.astype(np.float32))
    w_scaled_t = (w_q.astype(np.float32) * scale[:, None]).T
    w_fp8 = np.ascontiguousarray(w_scaled_t).astype(ml_dtypes.float8_e4m3fn)
    assert np.array_equal(w_fp8.astype(np.float32), w_scaled_t), \
        "scaled ternary weights must be exact in fp8e4m3"
    bias_bcast = np.ascontiguousarray(
        np.broadcast_to(bias.astype(np.float32), (P, OUT)))
    # Contraction-major layout for the PE: x^T[b] = [IN, T], bf16.
    xT_bf16 = np.ascontiguousarray(
        x.astype(ml_dtypes.bfloat16).transpose(0, 2, 1))

    nc = _CACHE.get("nc")
    if nc is None:
        nc = _CACHE["nc"] = _build()

    in_maps = [
        {"x": xT_bf16[b], "w": w_fp8, "bias": bias_bcast} for b in range(B)
    ]

    trace = bool(int(os.environ.get("BITLIN_TRACE", "0")))
    if trace:
        _install_prof_shim()
    res = run_bass_kernel_spmd(nc, in_maps, list(range(NCORES)), trace=trace)
    last_exec_time_ns = res.exec_time_ns

    out = np.stack([res.results[b]["y"] for b in range(B)], axis=0)
    return out.astype(np.float32, copy=False)
